# revision 2
# baseline (speedup 1.0000x reference)
"""Trainium2 Bass kernel for the VQ-codebook L2-embedding layer.

Forward math (first_n_real_mel == 0 path):
    p_code     = softmax(relu(temp) * (2*x@table.T - |table|^2))   per row
    new_latent = table[argmax(p_code)]                             (straight-through fwd)

Design:
  - Data-parallel over B: core i handles batches [4i, 4i+4) = 8192 rows.
  - GEMM: fp16 3-term split (xh*th + xl*th + xh*tl) at 1 cyc/row each,
    accumulated in fp32 PSUM -> fp32-grade accuracy at 3/4 the cost of fp32.
  - Softmax row-max/sum/argmax via DVE/ACT passes; codebook row fetch via
    indirect DMA gather.
"""

import os
import numpy as np

import concourse.bass as bass
import concourse.tile as tile
import concourse.bacc as bacc
from concourse import mybir
from concourse import bass_utils

F32 = mybir.dt.float32
F16 = mybir.dt.float16
I32 = mybir.dt.int32

B, S, D, V = 32, 2048, 256, 1024
NCORES = 8
RPC = (B * S) // NCORES          # rows per core = 8192
P = 128                          # partition tile height
NTILES = RPC // P                # 64
KCH = D // P                     # 2 K-chunks
SLAB = 512                       # rows loaded per input DMA slab
NSLAB = RPC // SLAB

# toggles resolved at build time (grading uses the defaults)
USE_STT = os.environ.get("USE_STT", "0") == "1"
USE_TTR = os.environ.get("USE_TTR", "0") == "1"
USE_ACT_ACCUM = os.environ.get("USE_ACT_ACCUM", "0") == "1"
USE_GATHER = os.environ.get("USE_GATHER", "1") == "1"

_cache = {}


def _build(sc: float):
    nc = bacc.Bacc("TRN2", target_bir_lowering=False, debug=False,
                   num_devices=NCORES)
    d_xh = nc.dram_tensor("xh", [P, KCH, RPC], F16, kind="ExternalInput")
    d_xl = nc.dram_tensor("xl", [P, KCH, RPC], F16, kind="ExternalInput")
    d_th = nc.dram_tensor("th", [P, KCH, V], F16, kind="ExternalInput")
    d_tl = nc.dram_tensor("tl", [P, KCH, V], F16, kind="ExternalInput")
    d_ysqb = nc.dram_tensor("ysqb", [P, V], F32, kind="ExternalInput")
    d_iotab = nc.dram_tensor("iotab", [P, V], F32, kind="ExternalInput")
    d_tbl = nc.dram_tensor("tbl", [V, D], F32, kind="ExternalInput")
    d_p = nc.dram_tensor("p", [RPC, V], F32, kind="ExternalOutput")
    d_lat = nc.dram_tensor("lat", [RPC, D], F32, kind="ExternalOutput")
    d_idx = nc.dram_tensor("idx", [RPC, 1], I32, kind="ExternalOutput")

    with tile.TileContext(nc) as tc:
        with tc.tile_pool(name="const", bufs=1) as cpool, \
             tc.tile_pool(name="xin", bufs=3) as xpool, \
             tc.tile_pool(name="work", bufs=3) as wpool, \
             tc.tile_pool(name="outp", bufs=3) as opool, \
             tc.tile_pool(name="small", bufs=4) as spool, \
             tc.tile_pool(name="ps", bufs=2, space="PSUM") as ps:

            s_th = cpool.tile([P, KCH, V], F16)
            nc.sync.dma_start(s_th[:], d_th.ap()[:])
            s_tl = cpool.tile([P, KCH, V], F16)
            nc.sync.dma_start(s_tl[:], d_tl.ap()[:])
            s_ysqb = cpool.tile([P, V], F32)
            nc.sync.dma_start(s_ysqb[:], d_ysqb.ap()[:])
            s_iotab = cpool.tile([P, V], F32)
            nc.sync.dma_start(s_iotab[:], d_iotab.ap()[:])

            for sl in range(NSLAB):
                r0 = sl * SLAB
                s_xh = xpool.tile([P, KCH, SLAB], F16, name=f"s_xh{sl}", tag="s_xh")
                nc.sync.dma_start(s_xh[:], d_xh.ap()[:, :, r0:r0 + SLAB])
                s_xl = xpool.tile([P, KCH, SLAB], F16, name=f"s_xl{sl}", tag="s_xl")
                nc.sync.dma_start(s_xl[:], d_xl.ap()[:, :, r0:r0 + SLAB])

                for j in range(SLAB // P):
                    row0 = r0 + j * P
                    c0, c1 = j * P, (j + 1) * P
                    psum = ps.tile([P, V], F32, name=f"psum{row0}", tag="psum")
                    for h in range(2):
                        first = True
                        for k in range(KCH):
                            for (xa, ta) in ((s_xh, s_th), (s_xl, s_th), (s_xh, s_tl)):
                                nc.tensor.matmul(
                                    psum[:, 512 * h:512 * (h + 1)],
                                    xa[:, k, c0:c1],
                                    ta[:, k, 512 * h:512 * (h + 1)],
                                    start=first,
                                    stop=(k == KCH - 1 and ta is s_tl),
                                )
                                first = False

                    t_sb = wpool.tile([P, V], F32, name=f"t{row0}", tag="t")
                    m_sb = spool.tile([P, 1], F32, name=f"m{row0}", tag="m")
                    if USE_TTR:
                        nc.vector.tensor_tensor_reduce(
                            out=t_sb[:], in0=psum[:], in1=s_ysqb[:],
                            scale=1.0, scalar=-3.0e38,
                            op0=mybir.AluOpType.subtract,
                            op1=mybir.AluOpType.max, accum_out=m_sb[:])
                    else:
                        nc.vector.tensor_tensor(
                            out=t_sb[:], in0=psum[:], in1=s_ysqb[:],
                            op=mybir.AluOpType.subtract)
                        nc.vector.reduce_max(m_sb[:], t_sb[:],
                                             axis=mybir.AxisListType.X)

                    nm_sb = spool.tile([P, 1], F32, name=f"nm{row0}", tag="nm")
                    nc.vector.tensor_scalar_mul(nm_sb[:], m_sb[:], -sc)

                    e_sb = wpool.tile([P, V], F32, name=f"e{row0}", tag="e")
                    s_sb = spool.tile([P, 1], F32, name=f"s{row0}", tag="s")
                    if USE_ACT_ACCUM:
                        nc.scalar.activation(
                            e_sb[:], t_sb[:], mybir.ActivationFunctionType.Exp,
                            bias=nm_sb[:], scale=sc, accum_out=s_sb[:])
                    else:
                        nc.scalar.activation(
                            e_sb[:], t_sb[:], mybir.ActivationFunctionType.Exp,
                            bias=nm_sb[:], scale=sc)
                        nc.vector.reduce_sum(s_sb[:], e_sb[:],
                                             axis=mybir.AxisListType.X)
                    r_sb = spool.tile([P, 1], F32, name=f"r{row0}", tag="r")
                    nc.vector.reciprocal(r_sb[:], s_sb[:])

                    idxf_sb = spool.tile([P, 1], F32, name=f"if{row0}", tag="if")
                    junk = wpool.tile([P, V], F32, name=f"j{row0}", tag="j")
                    if USE_STT:
                        nc.vector.scalar_tensor_tensor(
                            out=junk[:], in0=t_sb[:], scalar=m_sb[:],
                            in1=s_iotab[:], op0=mybir.AluOpType.is_equal,
                            op1=mybir.AluOpType.mult, accum_out=idxf_sb[:])
                    else:
                        nc.vector.tensor_scalar(
                            out=junk[:], in0=t_sb[:], scalar1=m_sb[:],
                            scalar2=None, op0=mybir.AluOpType.is_equal)
                        nc.vector.tensor_tensor(
                            out=junk[:], in0=junk[:], in1=s_iotab[:],
                            op=mybir.AluOpType.mult)
                        nc.vector.reduce_sum(idxf_sb[:], junk[:],
                                             axis=mybir.AxisListType.X)
                    idxi_sb = spool.tile([P, 1], I32, name=f"ii{row0}", tag="ii")
                    nc.vector.tensor_copy(idxi_sb[:], idxf_sb[:])
                    nc.sync.dma_start(d_idx.ap()[row0:row0 + P, :], idxi_sb[:])

                    p_sb = opool.tile([P, V], F32, name=f"p{row0}", tag="p")
                    nc.scalar.activation(
                        p_sb[:], e_sb[:], mybir.ActivationFunctionType.Copy,
                        bias=0.0, scale=r_sb[:])
                    nc.sync.dma_start(d_p.ap()[row0:row0 + P, :], p_sb[:])

                    if USE_GATHER:
                        g_sb = opool.tile([P, D], F32, name=f"g{row0}", tag="g")
                        nc.gpsimd.indirect_dma_start(
                            out=g_sb[:], out_offset=None,
                            in_=d_tbl.ap()[:],
                            in_offset=bass.IndirectOffsetOnAxis(
                                ap=idxi_sb[:, :1], axis=0),
                            bounds_check=V - 1, oob_is_err=False)
                        nc.sync.dma_start(d_lat.ap()[row0:row0 + P, :], g_sb[:])

    nc.compile()
    return nc


def kernel(enc_embs, table, temp, first_n_real_mel=None, _trace=False, **_kw):
    enc = np.ascontiguousarray(np.asarray(enc_embs, dtype=np.float32))
    tbl = np.ascontiguousarray(np.asarray(table, dtype=np.float32))
    sc = float(max(np.float32(np.asarray(temp).reshape(-1)[0]), np.float32(0)))

    if sc == 0.0:  # softmax of zeros: uniform p, argmax = 0 everywhere
        p = np.full((B, S, V), np.float32(1.0) / V, dtype=np.float32)
        lat = np.broadcast_to(tbl[0], (B, S, D)).astype(np.float32)
        return p, lat

    # host-side constant prep
    t2 = (2.0 * tbl.T).astype(np.float32)                       # [D, V]
    t2T = np.ascontiguousarray(
        t2.reshape(KCH, P, V).transpose(1, 0, 2))               # [P, KCH, V]
    th = t2T.astype(np.float16)
    tl = (t2T - th.astype(np.float32)).astype(np.float16)
    ysq = ((tbl.astype(np.float64)) ** 2).sum(1).astype(np.float32)
    ysqb = np.ascontiguousarray(np.broadcast_to(ysq, (P, V)))
    iotab = np.ascontiguousarray(
        np.broadcast_to(np.arange(V, dtype=np.float32), (P, V)))

    if sc not in _cache:
        _cache[sc] = _build(sc)
    nc = _cache[sc]

    # per-core input shards
    x = enc.reshape(B * S, D)
    in_maps = []
    for c in range(NCORES):
        xc = x[c * RPC:(c + 1) * RPC]                           # [RPC, D]
        xT = np.ascontiguousarray(
            xc.T.reshape(KCH, P, RPC).transpose(1, 0, 2))       # [P, KCH, RPC]
        xh = xT.astype(np.float16)
        xl = (xT - xh.astype(np.float32)).astype(np.float16)
        in_maps.append({"xh": xh, "xl": xl, "th": th, "tl": tl,
                        "ysqb": ysqb, "iotab": iotab, "tbl": tbl})

    try:
        res = bass_utils.run_bass_kernel_spmd(
            nc, in_maps, core_ids=list(range(NCORES)), trace=_trace)
    except ModuleNotFoundError:
        res = bass_utils.run_bass_kernel_spmd(
            nc, in_maps, core_ids=list(range(NCORES)), trace=False)

    p = np.empty((B * S, V), dtype=np.float32)
    lat = np.empty((B * S, D), dtype=np.float32)
    for c in range(NCORES):
        p[c * RPC:(c + 1) * RPC] = res.results[c]["p"]
        if USE_GATHER:
            lat[c * RPC:(c + 1) * RPC] = res.results[c]["lat"]
        else:
            idx = res.results[c]["idx"][:, 0].astype(np.int64)
            lat[c * RPC:(c + 1) * RPC] = tbl[idx]
    out = (p.reshape(B, S, V), lat.reshape(B, S, D))
    if _trace:
        kernel.last_exec_time_ns = res.exec_time_ns
    return out


# revision 5
# speedup vs baseline: 1.1536x; 1.1536x over previous
"""Trainium2 Bass kernel for the VQ-codebook L2-embedding layer.

Forward math (first_n_real_mel == 0 path):
    p_code     = softmax(relu(temp) * (2*x@table.T - |table|^2))   per row
    new_latent = table[argmax(p_code)]                             (straight-through fwd)

Design:
  - Data-parallel over B: core i handles batches [4i, 4i+4) = 8192 rows.
  - GEMM: fp16 3-term split (xh*th + xl*th + xh*tl) at 1 cyc/row each,
    accumulated in fp32 PSUM -> fp32-grade accuracy at 3/4 the cost of fp32.
  - Softmax row-max/sum/argmax via DVE/ACT passes; codebook row fetch via
    indirect DMA gather.
"""

import os
import numpy as np

import concourse.bass as bass
import concourse.tile as tile
import concourse.bacc as bacc
from concourse import mybir
from concourse import bass_utils

F32 = mybir.dt.float32
F16 = mybir.dt.float16
I32 = mybir.dt.int32

B, S, D, V = 32, 2048, 256, 1024
NCORES = 8
RPC = (B * S) // NCORES          # rows per core = 8192
P = 128                          # partition tile height
NTILES = RPC // P                # 64
KCH = D // P                     # 2 K-chunks
SLAB = 512                       # rows loaded per input DMA slab
NSLAB = RPC // SLAB

# toggles resolved at build time (grading uses the defaults)
USE_STT = os.environ.get("USE_STT", "1") == "1"
USE_TTR = os.environ.get("USE_TTR", "0") == "1"
USE_ACT_ACCUM = os.environ.get("USE_ACT_ACCUM", "1") == "1"
USE_GATHER = os.environ.get("USE_GATHER", "1") == "1"
USE_YFOLD = os.environ.get("USE_YFOLD", "1") == "1"

_cache = {}


def _build(sc: float):
    nc = bacc.Bacc("TRN2", target_bir_lowering=False, debug=False,
                   num_devices=NCORES)
    d_xh = nc.dram_tensor("xh", [P, KCH, RPC], F16, kind="ExternalInput")
    d_xl = nc.dram_tensor("xl", [P, KCH, RPC], F16, kind="ExternalInput")
    d_th = nc.dram_tensor("th", [P, KCH, V], F16, kind="ExternalInput")
    d_tl = nc.dram_tensor("tl", [P, KCH, V], F16, kind="ExternalInput")
    d_ysqb = nc.dram_tensor("ysqb", [P, V], F32, kind="ExternalInput")
    d_ones2 = nc.dram_tensor("ones2", [2, P], F16, kind="ExternalInput")
    d_yn2 = nc.dram_tensor("yn2", [2, V], F16, kind="ExternalInput")
    d_iotab = nc.dram_tensor("iotab", [P, V], F32, kind="ExternalInput")
    d_tbl = nc.dram_tensor("tbl", [V, D], F32, kind="ExternalInput")
    d_p = nc.dram_tensor("p", [RPC, V], F32, kind="ExternalOutput")
    d_lat = nc.dram_tensor("lat", [RPC, D], F32, kind="ExternalOutput")
    d_idx = nc.dram_tensor("idx", [RPC, 1], I32, kind="ExternalOutput")

    with tile.TileContext(nc) as tc:
        with tc.tile_pool(name="const", bufs=1) as cpool, \
             tc.tile_pool(name="xin", bufs=3) as xpool, \
             tc.tile_pool(name="work", bufs=3) as wpool, \
             tc.tile_pool(name="outp", bufs=3) as opool, \
             tc.tile_pool(name="small", bufs=4) as spool, \
             tc.tile_pool(name="ps", bufs=3, space="PSUM") as ps:

            s_th = cpool.tile([P, KCH, V], F16)
            nc.sync.dma_start(s_th[:], d_th.ap()[:])
            s_tl = cpool.tile([P, KCH, V], F16)
            nc.sync.dma_start(s_tl[:], d_tl.ap()[:])
            if USE_YFOLD:
                s_ones2 = cpool.tile([2, P], F16)
                nc.sync.dma_start(s_ones2[:], d_ones2.ap()[:])
                s_yn2 = cpool.tile([2, V], F16)
                nc.sync.dma_start(s_yn2[:], d_yn2.ap()[:])
            else:
                s_ysqb = cpool.tile([P, V], F32)
                nc.sync.dma_start(s_ysqb[:], d_ysqb.ap()[:])
            s_iotab = cpool.tile([P, V], F32)
            nc.sync.dma_start(s_iotab[:], d_iotab.ap()[:])

            for sl in range(NSLAB):
                r0 = sl * SLAB
                s_xh = xpool.tile([P, KCH, SLAB], F16, name=f"s_xh{sl}", tag="s_xh")
                nc.sync.dma_start(s_xh[:], d_xh.ap()[:, :, r0:r0 + SLAB])
                s_xl = xpool.tile([P, KCH, SLAB], F16, name=f"s_xl{sl}", tag="s_xl")
                nc.sync.dma_start(s_xl[:], d_xl.ap()[:, :, r0:r0 + SLAB])

                for j in range(SLAB // P):
                    row0 = r0 + j * P
                    c0, c1 = j * P, (j + 1) * P
                    psum = ps.tile([P, V], F32, name=f"psum{row0}", tag="psum")
                    for h in range(2):
                        first = True
                        if USE_YFOLD:
                            nc.tensor.matmul(
                                psum[:, 512 * h:512 * (h + 1)],
                                s_ones2[:, :],
                                s_yn2[:, 512 * h:512 * (h + 1)],
                                start=True, stop=False)
                            first = False
                        for k in range(KCH):
                            for (xa, ta) in ((s_xh, s_th), (s_xl, s_th), (s_xh, s_tl)):
                                nc.tensor.matmul(
                                    psum[:, 512 * h:512 * (h + 1)],
                                    xa[:, k, c0:c1],
                                    ta[:, k, 512 * h:512 * (h + 1)],
                                    start=first,
                                    stop=(k == KCH - 1 and ta is s_tl),
                                )
                                first = False

                    m_sb = spool.tile([P, 1], F32, name=f"m{row0}", tag="m")
                    if USE_YFOLD:
                        t_sb = psum
                        nc.vector.reduce_max(m_sb[:], psum[:],
                                             axis=mybir.AxisListType.X)
                    elif USE_TTR:
                        t_sb = wpool.tile([P, V], F32, name=f"t{row0}", tag="t")
                        nc.vector.tensor_tensor_reduce(
                            out=t_sb[:], in0=psum[:], in1=s_ysqb[:],
                            scale=1.0, scalar=-3.0e38,
                            op0=mybir.AluOpType.subtract,
                            op1=mybir.AluOpType.max, accum_out=m_sb[:])
                    else:
                        t_sb = wpool.tile([P, V], F32, name=f"t{row0}", tag="t")
                        nc.vector.tensor_tensor(
                            out=t_sb[:], in0=psum[:], in1=s_ysqb[:],
                            op=mybir.AluOpType.subtract)
                        nc.vector.reduce_max(m_sb[:], t_sb[:],
                                             axis=mybir.AxisListType.X)

                    nm_sb = spool.tile([P, 1], F32, name=f"nm{row0}", tag="nm")
                    nc.vector.tensor_scalar_mul(nm_sb[:], m_sb[:], -sc)

                    e_sb = wpool.tile([P, V], F32, name=f"e{row0}", tag="e")
                    s_sb = spool.tile([P, 1], F32, name=f"s{row0}", tag="s")
                    if USE_ACT_ACCUM:
                        nc.scalar.activation(
                            e_sb[:], t_sb[:], mybir.ActivationFunctionType.Exp,
                            bias=nm_sb[:], scale=sc, accum_out=s_sb[:])
                    else:
                        nc.scalar.activation(
                            e_sb[:], t_sb[:], mybir.ActivationFunctionType.Exp,
                            bias=nm_sb[:], scale=sc)
                        nc.vector.reduce_sum(s_sb[:], e_sb[:],
                                             axis=mybir.AxisListType.X)
                    r_sb = spool.tile([P, 1], F32, name=f"r{row0}", tag="r")
                    nc.vector.reciprocal(r_sb[:], s_sb[:])

                    idxf_sb = spool.tile([P, 1], F32, name=f"if{row0}", tag="if")
                    junk = wpool.tile([P, V], F32, name=f"j{row0}", tag="j")
                    if USE_STT:
                        nc.vector.scalar_tensor_tensor(
                            out=junk[:], in0=t_sb[:], scalar=m_sb[:],
                            in1=s_iotab[:], op0=mybir.AluOpType.is_equal,
                            op1=mybir.AluOpType.mult, accum_out=idxf_sb[:])
                    else:
                        nc.vector.tensor_scalar(
                            out=junk[:], in0=t_sb[:], scalar1=m_sb[:],
                            scalar2=None, op0=mybir.AluOpType.is_equal)
                        nc.vector.tensor_tensor(
                            out=junk[:], in0=junk[:], in1=s_iotab[:],
                            op=mybir.AluOpType.mult)
                        nc.vector.reduce_sum(idxf_sb[:], junk[:],
                                             axis=mybir.AxisListType.X)
                    idxi_sb = spool.tile([P, 1], I32, name=f"ii{row0}", tag="ii")
                    nc.vector.tensor_copy(idxi_sb[:], idxf_sb[:])
                    nc.sync.dma_start(d_idx.ap()[row0:row0 + P, :], idxi_sb[:])

                    p_sb = opool.tile([P, V], F32, name=f"p{row0}", tag="p")
                    nc.scalar.activation(
                        p_sb[:], e_sb[:], mybir.ActivationFunctionType.Copy,
                        bias=0.0, scale=r_sb[:])
                    nc.sync.dma_start(d_p.ap()[row0:row0 + P, :], p_sb[:])

                    if USE_GATHER:
                        g_sb = opool.tile([P, D], F32, name=f"g{row0}", tag="g")
                        nc.gpsimd.indirect_dma_start(
                            out=g_sb[:], out_offset=None,
                            in_=d_tbl.ap()[:],
                            in_offset=bass.IndirectOffsetOnAxis(
                                ap=idxi_sb[:, :1], axis=0),
                            bounds_check=V - 1, oob_is_err=False)
                        nc.sync.dma_start(d_lat.ap()[row0:row0 + P, :], g_sb[:])

    nc.compile()
    return nc


def kernel(enc_embs, table, temp, first_n_real_mel=None, _trace=False, **_kw):
    enc = np.ascontiguousarray(np.asarray(enc_embs, dtype=np.float32))
    tbl = np.ascontiguousarray(np.asarray(table, dtype=np.float32))
    sc = float(max(np.float32(np.asarray(temp).reshape(-1)[0]), np.float32(0)))

    if sc == 0.0:  # softmax of zeros: uniform p, argmax = 0 everywhere
        p = np.full((B, S, V), np.float32(1.0) / V, dtype=np.float32)
        lat = np.broadcast_to(tbl[0], (B, S, D)).astype(np.float32)
        return p, lat

    # host-side constant prep
    t2 = (2.0 * tbl.T).astype(np.float32)                       # [D, V]
    t2T = np.ascontiguousarray(
        t2.reshape(KCH, P, V).transpose(1, 0, 2))               # [P, KCH, V]
    th = t2T.astype(np.float16)
    tl = (t2T - th.astype(np.float32)).astype(np.float16)
    ysq = ((tbl.astype(np.float64)) ** 2).sum(1).astype(np.float32)
    ysqb = np.ascontiguousarray(np.broadcast_to(ysq, (P, V)))
    yn = -ysq
    ynh = yn.astype(np.float16)
    ynl = (yn - ynh.astype(np.float32)).astype(np.float16)
    yn2 = np.ascontiguousarray(np.stack([ynh, ynl]))          # [2, V]
    ones2 = np.ones((2, P), dtype=np.float16)
    iotab = np.ascontiguousarray(
        np.broadcast_to(np.arange(V, dtype=np.float32), (P, V)))

    if sc not in _cache:
        _cache[sc] = _build(sc)
    nc = _cache[sc]

    # per-core input shards
    x = enc.reshape(B * S, D)
    in_maps = []
    for c in range(NCORES):
        xc = x[c * RPC:(c + 1) * RPC]                           # [RPC, D]
        xT = np.ascontiguousarray(
            xc.T.reshape(KCH, P, RPC).transpose(1, 0, 2))       # [P, KCH, RPC]
        xh = xT.astype(np.float16)
        xl = (xT - xh.astype(np.float32)).astype(np.float16)
        in_maps.append({"xh": xh, "xl": xl, "th": th, "tl": tl,
                        "ysqb": ysqb, "iotab": iotab, "tbl": tbl,
                        "ones2": ones2, "yn2": yn2})

    try:
        res = bass_utils.run_bass_kernel_spmd(
            nc, in_maps, core_ids=list(range(NCORES)), trace=_trace)
    except ModuleNotFoundError:
        res = bass_utils.run_bass_kernel_spmd(
            nc, in_maps, core_ids=list(range(NCORES)), trace=False)

    p = np.empty((B * S, V), dtype=np.float32)
    lat = np.empty((B * S, D), dtype=np.float32)
    for c in range(NCORES):
        p[c * RPC:(c + 1) * RPC] = res.results[c]["p"]
        if USE_GATHER:
            lat[c * RPC:(c + 1) * RPC] = res.results[c]["lat"]
        else:
            idx = res.results[c]["idx"][:, 0].astype(np.int64)
            lat[c * RPC:(c + 1) * RPC] = tbl[idx]
    out = (p.reshape(B, S, V), lat.reshape(B, S, D))
    if _trace:
        kernel.last_exec_time_ns = res.exec_time_ns
    return out


# revision 6
# speedup vs baseline: 1.1545x; 1.0008x over previous
"""Trainium2 Bass kernel for the VQ-codebook L2-embedding layer.

Forward math (first_n_real_mel == 0 path):
    p_code     = softmax(relu(temp) * (2*x@table.T - |table|^2))   per row
    new_latent = table[argmax(p_code)]                             (straight-through fwd)

Design:
  - Data-parallel over B: core i handles batches [4i, 4i+4) = 8192 rows.
  - GEMM: fp16 3-term split (xh*th + xl*th + xh*tl) at 1 cyc/row each,
    accumulated in fp32 PSUM -> fp32-grade accuracy at 3/4 the cost of fp32.
  - Softmax row-max/sum/argmax via DVE/ACT passes; codebook row fetch via
    indirect DMA gather.
"""

import os
import numpy as np

import concourse.bass as bass
import concourse.tile as tile
import concourse.bacc as bacc
from concourse import mybir
from concourse import bass_utils

F32 = mybir.dt.float32
F16 = mybir.dt.float16
I32 = mybir.dt.int32

B, S, D, V = 32, 2048, 256, 1024
NCORES = 8
RPC = (B * S) // NCORES          # rows per core = 8192
P = 128                          # partition tile height
NTILES = RPC // P                # 64
KCH = D // P                     # 2 K-chunks
SLAB = 512                       # rows loaded per input DMA slab
NSLAB = RPC // SLAB

# toggles resolved at build time (grading uses the defaults)
USE_STT = os.environ.get("USE_STT", "1") == "1"
USE_TTR = os.environ.get("USE_TTR", "0") == "1"
USE_ACT_ACCUM = os.environ.get("USE_ACT_ACCUM", "1") == "1"
USE_GATHER = os.environ.get("USE_GATHER", "1") == "1"
USE_YFOLD = os.environ.get("USE_YFOLD", "1") == "1"

_cache = {}


def _build(sc: float):
    nc = bacc.Bacc("TRN2", target_bir_lowering=False, debug=False,
                   num_devices=NCORES)
    d_xh = nc.dram_tensor("xh", [P, KCH, RPC], F16, kind="ExternalInput")
    d_xl = nc.dram_tensor("xl", [P, KCH, RPC], F16, kind="ExternalInput")
    d_th = nc.dram_tensor("th", [P, KCH, V], F16, kind="ExternalInput")
    d_tl = nc.dram_tensor("tl", [P, KCH, V], F16, kind="ExternalInput")
    d_ysqb = nc.dram_tensor("ysqb", [P, V], F32, kind="ExternalInput")
    d_ones2 = nc.dram_tensor("ones2", [2, P], F16, kind="ExternalInput")
    d_yn2 = nc.dram_tensor("yn2", [2, V], F16, kind="ExternalInput")
    d_iotab = nc.dram_tensor("iotab", [P, V], F32, kind="ExternalInput")
    d_tbl = nc.dram_tensor("tbl", [V, D], F32, kind="ExternalInput")
    d_p = nc.dram_tensor("p", [RPC, V], F32, kind="ExternalOutput")
    d_lat = nc.dram_tensor("lat", [RPC, D], F32, kind="ExternalOutput")
    d_idx = nc.dram_tensor("idx", [RPC, 1], I32, kind="ExternalOutput")

    with tile.TileContext(nc) as tc:
        with tc.tile_pool(name="const", bufs=1) as cpool, \
             tc.tile_pool(name="xin", bufs=3) as xpool, \
             tc.tile_pool(name="work", bufs=4) as wpool, \
             tc.tile_pool(name="outp", bufs=4) as opool, \
             tc.tile_pool(name="small", bufs=8) as spool, \
             tc.tile_pool(name="ps", bufs=4, space="PSUM") as ps:

            s_th = cpool.tile([P, KCH, V], F16)
            nc.sync.dma_start(s_th[:], d_th.ap()[:])
            s_tl = cpool.tile([P, KCH, V], F16)
            nc.sync.dma_start(s_tl[:], d_tl.ap()[:])
            if USE_YFOLD:
                s_ones2 = cpool.tile([2, P], F16)
                nc.sync.dma_start(s_ones2[:], d_ones2.ap()[:])
                s_yn2 = cpool.tile([2, V], F16)
                nc.sync.dma_start(s_yn2[:], d_yn2.ap()[:])
            else:
                s_ysqb = cpool.tile([P, V], F32)
                nc.sync.dma_start(s_ysqb[:], d_ysqb.ap()[:])
            s_iotab = cpool.tile([P, V], F32)
            nc.sync.dma_start(s_iotab[:], d_iotab.ap()[:])

            for sl in range(NSLAB):
                r0 = sl * SLAB
                s_xh = xpool.tile([P, KCH, SLAB], F16, name=f"s_xh{sl}", tag="s_xh")
                nc.sync.dma_start(s_xh[:], d_xh.ap()[:, :, r0:r0 + SLAB])
                s_xl = xpool.tile([P, KCH, SLAB], F16, name=f"s_xl{sl}", tag="s_xl")
                nc.sync.dma_start(s_xl[:], d_xl.ap()[:, :, r0:r0 + SLAB])

                for j in range(SLAB // P):
                    row0 = r0 + j * P
                    c0, c1 = j * P, (j + 1) * P
                    psum = ps.tile([P, V], F32, name=f"psum{row0}", tag="psum")
                    for h in range(2):
                        first = True
                        if USE_YFOLD:
                            nc.tensor.matmul(
                                psum[:, 512 * h:512 * (h + 1)],
                                s_ones2[:, :],
                                s_yn2[:, 512 * h:512 * (h + 1)],
                                start=True, stop=False)
                            first = False
                        for k in range(KCH):
                            for (xa, ta) in ((s_xh, s_th), (s_xl, s_th), (s_xh, s_tl)):
                                nc.tensor.matmul(
                                    psum[:, 512 * h:512 * (h + 1)],
                                    xa[:, k, c0:c1],
                                    ta[:, k, 512 * h:512 * (h + 1)],
                                    start=first,
                                    stop=(k == KCH - 1 and ta is s_tl),
                                )
                                first = False

                    m_sb = spool.tile([P, 1], F32, name=f"m{row0}", tag="m")
                    if USE_YFOLD:
                        t_sb = psum
                        nc.vector.reduce_max(m_sb[:], psum[:],
                                             axis=mybir.AxisListType.X)
                    elif USE_TTR:
                        t_sb = wpool.tile([P, V], F32, name=f"t{row0}", tag="t")
                        nc.vector.tensor_tensor_reduce(
                            out=t_sb[:], in0=psum[:], in1=s_ysqb[:],
                            scale=1.0, scalar=-3.0e38,
                            op0=mybir.AluOpType.subtract,
                            op1=mybir.AluOpType.max, accum_out=m_sb[:])
                    else:
                        t_sb = wpool.tile([P, V], F32, name=f"t{row0}", tag="t")
                        nc.vector.tensor_tensor(
                            out=t_sb[:], in0=psum[:], in1=s_ysqb[:],
                            op=mybir.AluOpType.subtract)
                        nc.vector.reduce_max(m_sb[:], t_sb[:],
                                             axis=mybir.AxisListType.X)

                    nm_sb = spool.tile([P, 1], F32, name=f"nm{row0}", tag="nm")
                    nc.vector.tensor_scalar_mul(nm_sb[:], m_sb[:], -sc)

                    e_sb = wpool.tile([P, V], F32, name=f"e{row0}", tag="e")
                    s_sb = spool.tile([P, 1], F32, name=f"s{row0}", tag="s")
                    if USE_ACT_ACCUM:
                        nc.scalar.activation(
                            e_sb[:], t_sb[:], mybir.ActivationFunctionType.Exp,
                            bias=nm_sb[:], scale=sc, accum_out=s_sb[:])
                    else:
                        nc.scalar.activation(
                            e_sb[:], t_sb[:], mybir.ActivationFunctionType.Exp,
                            bias=nm_sb[:], scale=sc)
                        nc.vector.reduce_sum(s_sb[:], e_sb[:],
                                             axis=mybir.AxisListType.X)
                    r_sb = spool.tile([P, 1], F32, name=f"r{row0}", tag="r")
                    nc.vector.reciprocal(r_sb[:], s_sb[:])

                    idxf_sb = spool.tile([P, 1], F32, name=f"if{row0}", tag="if")
                    junk = wpool.tile([P, V], F32, name=f"j{row0}", tag="j")
                    if USE_STT:
                        nc.vector.scalar_tensor_tensor(
                            out=junk[:], in0=t_sb[:], scalar=m_sb[:],
                            in1=s_iotab[:], op0=mybir.AluOpType.is_equal,
                            op1=mybir.AluOpType.mult, accum_out=idxf_sb[:])
                    else:
                        nc.vector.tensor_scalar(
                            out=junk[:], in0=t_sb[:], scalar1=m_sb[:],
                            scalar2=None, op0=mybir.AluOpType.is_equal)
                        nc.vector.tensor_tensor(
                            out=junk[:], in0=junk[:], in1=s_iotab[:],
                            op=mybir.AluOpType.mult)
                        nc.vector.reduce_sum(idxf_sb[:], junk[:],
                                             axis=mybir.AxisListType.X)
                    idxi_sb = spool.tile([P, 1], I32, name=f"ii{row0}", tag="ii")
                    nc.vector.tensor_copy(idxi_sb[:], idxf_sb[:])
                    nc.sync.dma_start(d_idx.ap()[row0:row0 + P, :], idxi_sb[:])

                    p_sb = opool.tile([P, V], F32, name=f"p{row0}", tag="p")
                    nc.scalar.activation(
                        p_sb[:], e_sb[:], mybir.ActivationFunctionType.Copy,
                        bias=0.0, scale=r_sb[:])
                    nc.sync.dma_start(d_p.ap()[row0:row0 + P, :], p_sb[:])

                    if USE_GATHER:
                        g_sb = opool.tile([P, D], F32, name=f"g{row0}", tag="g")
                        nc.gpsimd.indirect_dma_start(
                            out=g_sb[:], out_offset=None,
                            in_=d_tbl.ap()[:],
                            in_offset=bass.IndirectOffsetOnAxis(
                                ap=idxi_sb[:, :1], axis=0),
                            bounds_check=V - 1, oob_is_err=False)
                        nc.sync.dma_start(d_lat.ap()[row0:row0 + P, :], g_sb[:])

    nc.compile()
    return nc


def kernel(enc_embs, table, temp, first_n_real_mel=None, _trace=False, **_kw):
    enc = np.ascontiguousarray(np.asarray(enc_embs, dtype=np.float32))
    tbl = np.ascontiguousarray(np.asarray(table, dtype=np.float32))
    sc = float(max(np.float32(np.asarray(temp).reshape(-1)[0]), np.float32(0)))

    if sc == 0.0:  # softmax of zeros: uniform p, argmax = 0 everywhere
        p = np.full((B, S, V), np.float32(1.0) / V, dtype=np.float32)
        lat = np.broadcast_to(tbl[0], (B, S, D)).astype(np.float32)
        return p, lat

    # host-side constant prep
    t2 = (2.0 * tbl.T).astype(np.float32)                       # [D, V]
    t2T = np.ascontiguousarray(
        t2.reshape(KCH, P, V).transpose(1, 0, 2))               # [P, KCH, V]
    th = t2T.astype(np.float16)
    tl = (t2T - th.astype(np.float32)).astype(np.float16)
    ysq = ((tbl.astype(np.float64)) ** 2).sum(1).astype(np.float32)
    ysqb = np.ascontiguousarray(np.broadcast_to(ysq, (P, V)))
    yn = -ysq
    ynh = yn.astype(np.float16)
    ynl = (yn - ynh.astype(np.float32)).astype(np.float16)
    yn2 = np.ascontiguousarray(np.stack([ynh, ynl]))          # [2, V]
    ones2 = np.ones((2, P), dtype=np.float16)
    iotab = np.ascontiguousarray(
        np.broadcast_to(np.arange(V, dtype=np.float32), (P, V)))

    if sc not in _cache:
        _cache[sc] = _build(sc)
    nc = _cache[sc]

    # per-core input shards
    x = enc.reshape(B * S, D)
    in_maps = []
    for c in range(NCORES):
        xc = x[c * RPC:(c + 1) * RPC]                           # [RPC, D]
        xT = np.ascontiguousarray(
            xc.T.reshape(KCH, P, RPC).transpose(1, 0, 2))       # [P, KCH, RPC]
        xh = xT.astype(np.float16)
        xl = (xT - xh.astype(np.float32)).astype(np.float16)
        in_maps.append({"xh": xh, "xl": xl, "th": th, "tl": tl,
                        "ysqb": ysqb, "iotab": iotab, "tbl": tbl,
                        "ones2": ones2, "yn2": yn2})

    try:
        res = bass_utils.run_bass_kernel_spmd(
            nc, in_maps, core_ids=list(range(NCORES)), trace=_trace)
    except ModuleNotFoundError:
        res = bass_utils.run_bass_kernel_spmd(
            nc, in_maps, core_ids=list(range(NCORES)), trace=False)

    p = np.empty((B * S, V), dtype=np.float32)
    lat = np.empty((B * S, D), dtype=np.float32)
    for c in range(NCORES):
        p[c * RPC:(c + 1) * RPC] = res.results[c]["p"]
        if USE_GATHER:
            lat[c * RPC:(c + 1) * RPC] = res.results[c]["lat"]
        else:
            idx = res.results[c]["idx"][:, 0].astype(np.int64)
            lat[c * RPC:(c + 1) * RPC] = tbl[idx]
    out = (p.reshape(B, S, V), lat.reshape(B, S, D))
    if _trace:
        kernel.last_exec_time_ns = res.exec_time_ns
    return out
